# revision 18
# baseline (speedup 1.0000x reference)
"""Trainium2 Bass kernel for nn_ModelNew_3556232921999.

Pipeline: ConvTranspose3d(16->32, k=3, s=2, p=1, op=1) -> MaxPool3d(2)
          -> softmax(ch) -> subtract -> swish -> max(ch)

Algebraic structure:
  * convT(stride 2) + maxpool(2) => pooled[c, m] = max over 8 parity classes,
    each a {0,1}^3-offset tap-conv of x. One matmul per 128 positions:
      lhsT = x-stack block [K=128=(od,oh,ow,cin), M=128 positions] (stationary)
      rhs  = W            [K=128, N=256=(c,parity)]                (moving)
    PSUM columns ordered (c outer, parity inner) so the parity-max is an
    innermost-axis reduce.
  * max_c silu(v_c) = max(silu(max_c v), silu(min_c v)) (silu quasiconvex).

V2 (from baseline trace: Vector 365us busy / 365us wait was the bottleneck):
  * stage-1 parity-max split across THREE engines working directly on PSUM:
      DVE  tensor_reduce(X)  channels [0, C0)
      Pool pairwise max tree channels [C0, C1)
      Act  copy-evacuate     channels [C1, 32) + DVE bf16 4x fold
  * softmax divide via reciprocal + DMA stride-0 broadcast (DMA is idle)
  * all epilogue ops batched per 8-block group (1024 positions)
  * silu tail once per batch-slice at the very end (2 ACT table swaps total)

Sharding: data-parallel over batch B=16 -> 2 per core x 8 cores.
"""

import os
import sys

sys.path.insert(0, "/opt/trn_rl_repo")

import numpy as np
import ml_dtypes

# ---------------------------------------------------------------- constants
IN_C, OUT_C, K, STRIDE, PAD, OUT_PAD = 16, 32, 3, 2, 1, 1
B, D, H, W = 16, 16, 64, 64
N_CORES = 8
B_PER_CORE = B // N_CORES  # 2

PLANE = H * W            # 4096 positions per (b, d) plane
BLK = 128                # positions per matmul block
BLKS_PER_PLANE = PLANE // BLK      # 32
GRP = 8                  # matmul blocks per psum group (1024 positions)
GRPS_PER_PLANE = BLKS_PER_PLANE // GRP  # 4

# stage-1 channel split: [0,C0) DVE reduce from PSUM, [C0,32) Act evacuate
# then DVE folds the evacuation (bf16 4x). Pool/GpSimd can neither access
# PSUM nor execute max ops on this target, so it only gets the arithmetic
# stages (bias add, softmax multiply, subtract).
C0 = 12

X_NP_DT = ml_dtypes.bfloat16
W_NP_DT = ml_dtypes.bfloat16

_COMPILED = {}
LAST_EXEC_NS = None
LAST_RESULTS = None


def _tap(o, p):
    """Kernel tap index used by parity class p at window offset o, or None."""
    if p == 0:
        return 1 if o == 0 else None
    return 2 if o == 0 else 0


def build_wrhs(weight):
    """[128 rows=(od,oh,ow,cin), 256 cols=(c, pd,ph,pw)] conv matrix."""
    wr = np.zeros((2, 2, 2, IN_C, OUT_C, 2, 2, 2), dtype=np.float32)
    for od in range(2):
        for oh in range(2):
            for ow in range(2):
                for pd in range(2):
                    kd = _tap(od, pd)
                    if kd is None:
                        continue
                    for ph in range(2):
                        kh = _tap(oh, ph)
                        if kh is None:
                            continue
                        for pw in range(2):
                            kw = _tap(ow, pw)
                            if kw is None:
                                continue
                            # weight: [cin, cout, kd, kh, kw]
                            wr[od, oh, ow, :, :, pd, ph, pw] = weight[:, :, kd, kh, kw]
    return wr.reshape(128, 256)


def build_xstack(x):
    """[B, D, 128 rows=(od,oh,ow,cin), PLANE] shifted/padded copies of x."""
    xp = np.zeros((B, IN_C, D + 1, H + 1, W + 1), dtype=np.float32)
    xp[:, :, :D, :H, :W] = x
    S = np.empty((B, D, 2, 2, 2, IN_C, H, W), dtype=X_NP_DT)
    for od in range(2):
        for oh in range(2):
            for ow in range(2):
                sl = xp[:, :, od:od + D, oh:oh + H, ow:ow + W]
                S[:, :, od, oh, ow] = sl.transpose(0, 2, 1, 3, 4).astype(X_NP_DT)
    return S.reshape(B, D, 128, PLANE)


def build_kernel():
    from concourse import bass, bacc, mybir, tile

    f32 = mybir.dt.float32
    bf16 = mybir.dt.bfloat16
    x_dt = bf16
    w_dt = bf16
    Alu = mybir.AluOpType
    Act = mybir.ActivationFunctionType
    Ax = mybir.AxisListType

    nc = bacc.Bacc("TRN2", target_bir_lowering=False, debug=False,
                   num_devices=N_CORES)

    xs_h = nc.declare_dram_parameter("xs", [B_PER_CORE, D, 128, PLANE], x_dt,
                                     isOutput=False)
    wr_h = nc.declare_dram_parameter("wr", [128, 256], w_dt, isOutput=False)
    bias_h = nc.declare_dram_parameter("biasrep", [128, 256], bf16,
                                       isOutput=False)
    sub_h = nc.declare_dram_parameter("subrep", [128, 256], bf16,
                                      isOutput=False)
    y_h = nc.declare_dram_parameter("y", [B_PER_CORE, D, PLANE], f32,
                                    isOutput=True)

    NACT = OUT_C - C0    # channels evacuated by Act, folded by Pool

    from contextlib import ExitStack

    with tile.TileContext(nc) as tc:
        with ExitStack() as stack:
            pool_specs = dict(
                const=1, xslab=3, g1=2, g2=2, ev=3, pooled=3, pb=2, exp=2,
                z=2, r=2, rrep=2, m=2, v=2, ext=1, sil=1, ost=1)
            pools = {
                name: stack.enter_context(tc.tile_pool(name=name, bufs=n))
                for name, n in pool_specs.items()}
            pools["psum"] = stack.enter_context(
                tc.tile_pool(name="psum", bufs=2, space="PSUM"))
            constp, xpool, psump = (pools[k] for k in ("const", "xslab", "psum"))
            g1p, g2p, evp, plp, pbp = (
                pools[k] for k in ("g1", "g2", "ev", "pooled", "pb"))
            ep, zp, rp, rrepp, mp, vp = (
                pools[k] for k in ("exp", "z", "r", "rrep", "m", "v"))
            extp, silp, ostp = (pools[k] for k in ("ext", "sil", "ost"))
            wr = constp.tile([128, 256], w_dt)
            nc.sync.dma_start(wr[:], wr_h[:, :])
            biasrep = constp.tile([128, GRP, OUT_C], bf16)
            nc.sync.dma_start(
                biasrep[:].rearrange("p a b -> p (a b)"), bias_h[:, :])
            subrep = constp.tile([128, GRP, OUT_C], bf16)
            nc.sync.dma_start(
                subrep[:].rearrange("p a b -> p (a b)"), sub_h[:, :])

            def stage2(pooled, ext, col):
                """softmax -> subtract -> vmax/vmin for one group."""
                pb = pbp.tile([128, GRP, OUT_C], bf16, tag="pb")
                nc.gpsimd.tensor_tensor(pb[:], pooled[:], biasrep[:], Alu.add)
                E = ep.tile([128, GRP, OUT_C], bf16, tag="E")
                nc.scalar.activation(E[:], pb[:], Act.Exp)
                Z = zp.tile([128, GRP], f32, tag="Z")
                nc.vector.tensor_reduce(Z[:], E[:], axis=Ax.X, op=Alu.add)
                R = rp.tile([128, GRP], f32, tag="R")
                nc.vector.reciprocal(R[:], Z[:])
                m = mp.tile([128, GRP, OUT_C], bf16, tag="m")
                nc.gpsimd.tensor_tensor(
                    m[:], E[:],
                    R[:].unsqueeze(2).broadcast_to([128, GRP, OUT_C]),
                    Alu.mult)
                v = vp.tile([128, GRP, OUT_C], bf16, tag="v")
                nc.gpsimd.tensor_tensor(v[:], m[:], subrep[:], Alu.subtract)
                nc.vector.tensor_reduce(
                    ext[:, 0, col:col + GRP], v[:], axis=Ax.X, op=Alu.max)
                nc.vector.tensor_reduce(
                    ext[:, 1, col:col + GRP], v[:], axis=Ax.X, op=Alu.min)

            exts = []
            # stage-2 for group g is emitted one group LATE (software
            # pipelining): keeps the Act engine's in-order queue free of
            # head-of-line blocking (exp(g) would otherwise stall evac(g+1),
            # delaying PSUM release and starving the PE).
            pending = None
            for b in range(B_PER_CORE):
                # vmax/vmin staging: [128, 2(ismin), D*GRPS*GRP = 512]
                ext = extp.tile([128, 2, D * GRPS_PER_PLANE * GRP], f32,
                                tag=f"ext{b}")
                exts.append(ext)
                for d in range(D):
                    slab = xpool.tile([128, PLANE], x_dt, tag="slab")
                    nc.sync.dma_start(slab[:], xs_h[b, d])
                    for g in range(GRPS_PER_PLANE):
                        psum = psump.tile([128, GRP, OUT_C, 8], f32, tag="ps")
                        for k in range(GRP):
                            blk = (g * GRP + k) * BLK
                            nc.tensor.matmul(
                                psum[:, k].rearrange("p c q -> p (c q)"),
                                slab[:, blk:blk + BLK],
                                wr[:],
                                start=True, stop=True,
                            )
                        pooled = plp.tile([128, GRP, OUT_C], bf16, tag="pool")
                        # --- stage 1: parity max 8 -> 1 ---
                        nc.vector.tensor_reduce(
                            pooled[:, :, 0:C0], psum[:, :, 0:C0, :],
                            axis=Ax.X, op=Alu.max)
                        ev = evp.tile([128, GRP, NACT, 8], bf16, tag="ev")
                        nc.scalar.activation(
                            ev[:], psum[:, :, C0:OUT_C, :], Act.Copy)
                        nc.vector.tensor_reduce(
                            pooled[:, :, C0:OUT_C], ev[:], axis=Ax.X,
                            op=Alu.max)
                        if pending is not None:
                            stage2(*pending)
                        col = (d * GRPS_PER_PLANE + g) * GRP
                        pending = (pooled, ext, col)
            stage2(*pending)
            # ---- tail: tiny silu + final pairwise max, both b at once ----
            for b in range(B_PER_CORE):
                ext = exts[b]
                sil = silp.tile([128, 2, 512], f32, tag=f"sil{b}")
                nc.scalar.activation(
                    sil[:].rearrange("p a b -> p (a b)"),
                    ext[:].rearrange("p a b -> p (a b)"), Act.Silu)
                ost = ostp.tile([128, 512], f32, tag=f"ost{b}")
                nc.vector.tensor_tensor(ost[:], sil[:, 0, :], sil[:, 1, :],
                                        Alu.max)
                nc.sync.dma_start(
                    y_h[b].flatten().rearrange(
                        "(dd hg blk p) -> p dd hg blk",
                        dd=D, hg=GRPS_PER_PLANE, blk=GRP, p=BLK),
                    ost[:].rearrange("p (dd hg blk) -> p dd hg blk",
                                     dd=D, hg=GRPS_PER_PLANE, blk=GRP))

    nc.compile()
    return nc


def _get_nc():
    if "nc" not in _COMPILED:
        _COMPILED["nc"] = build_kernel()
    return _COMPILED["nc"]


def kernel(x, weight, bias, subtract):
    from concourse.bass_utils import run_bass_kernel_spmd

    x = np.asarray(x, dtype=np.float32)
    weight = np.asarray(weight, dtype=np.float32)
    bias = np.asarray(bias, dtype=np.float32)
    subtract = np.asarray(subtract, dtype=np.float32)

    nc = _get_nc()

    xs = build_xstack(x)                      # [B, D, 128, PLANE]
    wr = build_wrhs(weight).astype(W_NP_DT)   # [128, 256] cols (c, parity)
    # biasrep/subrep: [128, 256] = (8 groups x 32 ch) pattern, bf16
    biasrep = np.tile(bias[None, None, :], (128, GRP, 1)).reshape(128, 256)
    biasrep = biasrep.astype(ml_dtypes.bfloat16)
    subrep = np.tile(subtract[None, None, :], (128, GRP, 1)).reshape(
        128, 256).astype(ml_dtypes.bfloat16)

    in_maps = []
    for c in range(N_CORES):
        in_maps.append({
            "xs": np.ascontiguousarray(xs[c * B_PER_CORE:(c + 1) * B_PER_CORE]),
            "wr": wr,
            "biasrep": biasrep,
            "subrep": subrep,
        })

    kw = {}
    if os.environ.get("KERNEL_TRACE_DIR"):
        kw["tmpdir"] = os.environ["KERNEL_TRACE_DIR"]
    res = run_bass_kernel_spmd(nc, in_maps, core_ids=list(range(N_CORES)), **kw)
    global LAST_EXEC_NS, LAST_RESULTS
    LAST_EXEC_NS = res.exec_time_ns
    LAST_RESULTS = res
    outs = [res.results[c]["y"].reshape(B_PER_CORE, D, H, W)
            for c in range(N_CORES)]
    return np.concatenate(outs, axis=0)


# revision 24
# speedup vs baseline: 1.0292x; 1.0292x over previous
"""Trainium2 Bass kernel for nn_ModelNew_3556232921999.

Pipeline: ConvTranspose3d(16->32, k=3, s=2, p=1, op=1) -> MaxPool3d(2)
          -> softmax(ch) -> subtract -> swish -> max(ch)

Algebraic structure:
  * convT(stride 2) + maxpool(2) => pooled[c, m] = max over 8 parity classes,
    each a {0,1}^3-offset tap-conv of x. One matmul per 128 positions:
      lhsT = x-stack block [K=128=(od,oh,ow,cin), M=128 positions] (stationary)
      rhs  = W            [K=128, N=256=(c,parity)]                (moving)
    PSUM columns ordered (c outer, parity inner) so the parity-max is an
    innermost-axis reduce.
  * max_c silu(v_c) = max(silu(max_c v), silu(min_c v)) (silu quasiconvex).

V2 (from baseline trace: Vector 365us busy / 365us wait was the bottleneck):
  * stage-1 parity-max split across THREE engines working directly on PSUM:
      DVE  tensor_reduce(X)  channels [0, C0)
      Pool pairwise max tree channels [C0, C1)
      Act  copy-evacuate     channels [C1, 32) + DVE bf16 4x fold
  * softmax divide via reciprocal + DMA stride-0 broadcast (DMA is idle)
  * all epilogue ops batched per 8-block group (1024 positions)
  * silu tail once per batch-slice at the very end (2 ACT table swaps total)

Sharding: data-parallel over batch B=16 -> 2 per core x 8 cores.
"""

import os
import sys

sys.path.insert(0, "/opt/trn_rl_repo")

import numpy as np
import ml_dtypes

# ---------------------------------------------------------------- constants
IN_C, OUT_C, K, STRIDE, PAD, OUT_PAD = 16, 32, 3, 2, 1, 1
B, D, H, W = 16, 16, 64, 64
N_CORES = 8
B_PER_CORE = B // N_CORES  # 2

PLANE = H * W            # 4096 positions per (b, d) plane
BLK = 128                # positions per matmul block
BLKS_PER_PLANE = PLANE // BLK      # 32
HGRP = 4                 # matmul blocks per psum tile (2 PSUM banks, 4 bufs)
HGRPS_PER_PLANE = BLKS_PER_PLANE // HGRP  # 8
GRP = 8                  # blocks per stage-2 batch (2 psum tiles, 1024 pos)
GRPS_PER_PLANE = BLKS_PER_PLANE // GRP  # 4

X_NP_DT = ml_dtypes.bfloat16
W_NP_DT = ml_dtypes.bfloat16

_COMPILED = {}
LAST_EXEC_NS = None
LAST_RESULTS = None


def _tap(o, p):
    """Kernel tap index used by parity class p at window offset o, or None."""
    if p == 0:
        return 1 if o == 0 else None
    return 2 if o == 0 else 0


def build_wrhs(weight):
    """[128 rows=(od,oh,ow,cin), 256 cols=(pd,ph,pw,c)] conv matrix.

    Parity-major columns: the pool-window parity max folds as three
    pairwise column-half maxes (pd, then ph, then pw), each contiguous.
    """
    wr = np.zeros((2, 2, 2, IN_C, 2, 2, 2, OUT_C), dtype=np.float32)
    for od in range(2):
        for oh in range(2):
            for ow in range(2):
                for pd in range(2):
                    kd = _tap(od, pd)
                    if kd is None:
                        continue
                    for ph in range(2):
                        kh = _tap(oh, ph)
                        if kh is None:
                            continue
                        for pw in range(2):
                            kw = _tap(ow, pw)
                            if kw is None:
                                continue
                            # weight: [cin, cout, kd, kh, kw]
                            wr[od, oh, ow, :, pd, ph, pw, :] = weight[:, :, kd, kh, kw]
    return wr.reshape(128, 256)


def build_xstack(x):
    """[B, D, 128 rows=(od,oh,ow,cin), PLANE] shifted/padded copies of x."""
    xp = np.zeros((B, IN_C, D + 1, H + 1, W + 1), dtype=np.float32)
    xp[:, :, :D, :H, :W] = x
    S = np.empty((B, D, 2, 2, 2, IN_C, H, W), dtype=X_NP_DT)
    for od in range(2):
        for oh in range(2):
            for ow in range(2):
                sl = xp[:, :, od:od + D, oh:oh + H, ow:ow + W]
                S[:, :, od, oh, ow] = sl.transpose(0, 2, 1, 3, 4).astype(X_NP_DT)
    return S.reshape(B, D, 128, PLANE)


def build_kernel():
    from concourse import bass, bacc, mybir, tile

    f32 = mybir.dt.float32
    bf16 = mybir.dt.bfloat16
    x_dt = bf16
    w_dt = bf16
    Alu = mybir.AluOpType
    Act = mybir.ActivationFunctionType
    Ax = mybir.AxisListType

    nc = bacc.Bacc("TRN2", target_bir_lowering=False, debug=False,
                   num_devices=N_CORES)

    xs_h = nc.declare_dram_parameter("xs", [B_PER_CORE, D, 128, PLANE], x_dt,
                                     isOutput=False)
    wr_h = nc.declare_dram_parameter("wr", [128, 256], w_dt, isOutput=False)
    bias_h = nc.declare_dram_parameter("biasrep", [128, 256], bf16,
                                       isOutput=False)
    sub_h = nc.declare_dram_parameter("subrep", [128, 256], bf16,
                                      isOutput=False)
    y_h = nc.declare_dram_parameter("y", [B_PER_CORE, D, PLANE], f32,
                                    isOutput=True)

    from contextlib import ExitStack

    with tile.TileContext(nc) as tc:
        with ExitStack() as stack:
            pool_specs = dict(
                const=1, xslab=3, ev=3, t1=3, t2=3, pooled=3, pb=2, exp=2,
                z1=2, z2=2, z=2, r=2, m=2, v=2, ext=1, sil=1, ost=1)
            pools = {
                name: stack.enter_context(tc.tile_pool(name=name, bufs=n))
                for name, n in pool_specs.items()}
            pools["psum"] = stack.enter_context(
                tc.tile_pool(name="psum", bufs=4, space="PSUM"))
            constp, xpool, psump = (pools[k] for k in ("const", "xslab", "psum"))
            evp, t1p, t2p, plp, pbp = (
                pools[k] for k in ("ev", "t1", "t2", "pooled", "pb"))
            ep, z1p, z2p, zp, rp, mp, vp = (
                pools[k] for k in ("exp", "z1", "z2", "z", "r", "m", "v"))
            extp, silp, ostp = (pools[k] for k in ("ext", "sil", "ost"))
            wr = constp.tile([128, 256], w_dt)
            nc.sync.dma_start(wr[:], wr_h[:, :])
            biasrep = constp.tile([128, GRP, OUT_C], bf16)
            nc.sync.dma_start(
                biasrep[:].rearrange("p a b -> p (a b)"), bias_h[:, :])
            subrep = constp.tile([128, GRP, OUT_C], bf16)
            nc.sync.dma_start(
                subrep[:].rearrange("p a b -> p (a b)"), sub_h[:, :])

            def stage2(pooled, ext, col):
                """softmax -> subtract -> vmax/vmin for one 8-block group."""
                pb = pbp.tile([128, GRP, OUT_C], bf16, tag="pb")
                nc.vector.tensor_tensor(pb[:], pooled[:], biasrep[:], Alu.add)
                E = ep.tile([128, GRP, OUT_C], bf16, tag="E")
                nc.scalar.activation(E[:], pb[:], Act.Exp)
                # Z via Pool partial adds, DVE finishes the last 8
                z1 = z1p.tile([128, GRP, 16], f32, tag="z1")
                nc.gpsimd.tensor_tensor(z1[:], E[:, :, 0:16], E[:, :, 16:32],
                                        Alu.add)
                z2 = z2p.tile([128, GRP, 8], f32, tag="z2")
                nc.gpsimd.tensor_tensor(z2[:], z1[:, :, 0:8], z1[:, :, 8:16],
                                        Alu.add)
                Z = zp.tile([128, GRP], f32, tag="Z")
                nc.vector.tensor_reduce(Z[:], z2[:], axis=Ax.X, op=Alu.add)
                R = rp.tile([128, GRP], f32, tag="R")
                nc.vector.reciprocal(R[:], Z[:])
                m = mp.tile([128, GRP, OUT_C], bf16, tag="m")
                nc.gpsimd.tensor_tensor(
                    m[:], E[:],
                    R[:].unsqueeze(2).broadcast_to([128, GRP, OUT_C]),
                    Alu.mult)
                v = vp.tile([128, GRP, OUT_C], bf16, tag="v")
                nc.vector.tensor_tensor(v[:], m[:], subrep[:], Alu.subtract)
                nc.vector.tensor_reduce(
                    ext[:, 0, col:col + GRP], v[:], axis=Ax.X, op=Alu.max)
                nc.vector.tensor_reduce(
                    ext[:, 1, col:col + GRP], v[:], axis=Ax.X, op=Alu.min)

            exts = []
            # Half-group (4-block) PSUM tiles x 4 bufs give pipeline depth 4.
            # stage-2 runs per PAIR of half-groups, emitted one half-group
            # LATE (software pipelining: keeps Act's in-order queue free of
            # head-of-line blocking that would delay PSUM release).
            pending = None
            pooled = None
            for b in range(B_PER_CORE):
                # vmax/vmin staging: [128, 2(ismin), D*GRPS*GRP = 512]
                ext = extp.tile([128, 2, D * GRPS_PER_PLANE * GRP], f32,
                                tag=f"ext{b}")
                exts.append(ext)
                for d in range(D):
                    slab = xpool.tile([128, PLANE], x_dt, tag="slab")
                    nc.sync.dma_start(slab[:], xs_h[b, d])
                    for hg in range(HGRPS_PER_PLANE):
                        half = hg % 2
                        if half == 0:
                            pooled = plp.tile([128, GRP, OUT_C], bf16,
                                              tag="pool")
                        psum = psump.tile([128, HGRP, 256], f32, tag="ps")
                        for k in range(HGRP):
                            blk = (hg * HGRP + k) * BLK
                            nc.tensor.matmul(
                                psum[:, k],
                                slab[:, blk:blk + BLK],
                                wr[:],
                                start=True, stop=True,
                            )
                        # --- stage 1: parity max as 3 pairwise halves ---
                        # cols are (pd, ph, pw, c); fold pd, then ph, then pw
                        ev = evp.tile([128, HGRP, 128], bf16, tag="ev")
                        nc.scalar.activation(ev[:], psum[:, :, 128:256],
                                             Act.Copy)
                        t1 = t1p.tile([128, HGRP, 128], bf16, tag="t1")
                        nc.vector.tensor_tensor(t1[:], psum[:, :, 0:128],
                                                ev[:], Alu.max)
                        t2 = t2p.tile([128, HGRP, 64], bf16, tag="t2")
                        nc.vector.tensor_tensor(t2[:], t1[:, :, 0:64],
                                                t1[:, :, 64:128], Alu.max)
                        nc.vector.tensor_tensor(
                            pooled[:, half * HGRP:(half + 1) * HGRP, :],
                            t2[:, :, 0:32], t2[:, :, 32:64], Alu.max)
                        if half == 1:
                            if pending is not None:
                                stage2(*pending)
                            col = (d * HGRPS_PER_PLANE + hg - 1) * HGRP
                            pending = (pooled, ext, col)
            stage2(*pending)
            # ---- tail: tiny silu + final pairwise max, both b at once ----
            for b in range(B_PER_CORE):
                ext = exts[b]
                sil = silp.tile([128, 2, 512], f32, tag=f"sil{b}")
                nc.scalar.activation(
                    sil[:].rearrange("p a b -> p (a b)"),
                    ext[:].rearrange("p a b -> p (a b)"), Act.Silu)
                ost = ostp.tile([128, 512], f32, tag=f"ost{b}")
                nc.vector.tensor_tensor(ost[:], sil[:, 0, :], sil[:, 1, :],
                                        Alu.max)
                nc.sync.dma_start(
                    y_h[b].flatten().rearrange(
                        "(dd hg blk p) -> p dd hg blk",
                        dd=D, hg=GRPS_PER_PLANE, blk=GRP, p=BLK),
                    ost[:].rearrange("p (dd hg blk) -> p dd hg blk",
                                     dd=D, hg=GRPS_PER_PLANE, blk=GRP))

    nc.compile()
    return nc


def _get_nc():
    if "nc" not in _COMPILED:
        _COMPILED["nc"] = build_kernel()
    return _COMPILED["nc"]


def kernel(x, weight, bias, subtract):
    from concourse.bass_utils import run_bass_kernel_spmd

    x = np.asarray(x, dtype=np.float32)
    weight = np.asarray(weight, dtype=np.float32)
    bias = np.asarray(bias, dtype=np.float32)
    subtract = np.asarray(subtract, dtype=np.float32)

    nc = _get_nc()

    xs = build_xstack(x)                      # [B, D, 128, PLANE]
    wr = build_wrhs(weight).astype(W_NP_DT)   # [128, 256] cols (c, parity)
    # biasrep/subrep: [128, 256] = (8 groups x 32 ch) pattern, bf16
    biasrep = np.tile(bias[None, None, :], (128, GRP, 1)).reshape(128, 256)
    biasrep = biasrep.astype(ml_dtypes.bfloat16)
    subrep = np.tile(subtract[None, None, :], (128, GRP, 1)).reshape(
        128, 256).astype(ml_dtypes.bfloat16)

    in_maps = []
    for c in range(N_CORES):
        in_maps.append({
            "xs": np.ascontiguousarray(xs[c * B_PER_CORE:(c + 1) * B_PER_CORE]),
            "wr": wr,
            "biasrep": biasrep,
            "subrep": subrep,
        })

    kw = {}
    if os.environ.get("KERNEL_TRACE_DIR"):
        kw["tmpdir"] = os.environ["KERNEL_TRACE_DIR"]
    res = run_bass_kernel_spmd(nc, in_maps, core_ids=list(range(N_CORES)), **kw)
    global LAST_EXEC_NS, LAST_RESULTS
    LAST_EXEC_NS = res.exec_time_ns
    LAST_RESULTS = res
    outs = [res.results[c]["y"].reshape(B_PER_CORE, D, H, W)
            for c in range(N_CORES)]
    return np.concatenate(outs, axis=0)
